# revision 48
# baseline (speedup 1.0000x reference)
"""Trainium2 Bass kernel for nn_Attention_33595234189924.

Multi-head attention (B=2, S=2048, D=2048, H=16, hd=128) with RoPE,
tensor-parallel over heads: 8 cores x 2 heads each.

v4 schedule, three phases:
  1. chunks 0-3 (batch 0): full q/k/v projections, PE-dense; chunk 0 is
     split in two 256-token halves with per-kt interleaved weight/x DMAs
     for fast rampup.
  2. chunks 4-7 (batch 1): k+v projections only, with batch-0 attention
     segments (fully unlocked after phase 1) interleaved at ~18 units
     per chunk.  The Tile list-scheduler slots attention matmuls into
     projection-chain bubbles.
  3. deferred q chains for chunks 4-7 (both heads), each immediately
     followed by the batch-1 attention segment it unlocks.  This keeps
     ~27us of PE projection work in the final ACT-heavy region so the
     PE never starves while exp runs.
  PSUM in phases 2-3: proj(1 bank) + scores(2x2) + o(2) + r(1) = 8.

Per-core dataflow (activations in [feature, token] layout):
  - k/q projections -> PSUM -> +bias -> RoPE (rotate-half via host-side
    even/odd weight-row permutation + 64-partition block swap by DMA)
  - v projection in natural [token, hd] layout (xT tiles as stationary)
  - scores^T = k_tile^T @ q  per 128-key tile, exp on ACT (scale fused),
    probs kept transposed -> PV accumulates in PSUM; row-sums via
    all-ones stationary matmul (output pre-broadcast across partitions)
  - out = PV/rowsum + bv  (v-bias folded through softmax identity)
"""

import os
import sys
from collections import deque

sys.path.insert(0, "/opt/trn_rl_repo")

import numpy as np
import ml_dtypes

import concourse.bass as bass
import concourse.tile as tile
from concourse import bacc, mybir
from concourse.bass import ts
from concourse.bass_utils import run_bass_kernel_spmd

# If anything enables tracing (e.g. BASS_TRACE in the environment) and the
# image's antenv lacks axon_hooks, run_bass_kernel_spmd would crash on
# import. Register a null hook so it degrades to the untraced path.
try:
    from antenv import axon_hooks as _ah  # noqa: F401
except Exception:
    import types as _types

    _m = _types.ModuleType("antenv.axon_hooks")
    _m.get_axon_ntff_profile_hook = lambda: None
    _m.set_axon_ntff_profile_hook = lambda hook: None
    sys.modules["antenv.axon_hooks"] = _m

B, S, D, H = 2, 2048, 2048, 16
HD = 128
T = B * S
NCORES = 8
NKT = D // 128        # contraction tiles for projections
CHUNK = 512           # token chunk in projection phase
QCHUNK = 512          # query chunk in attention phase
NJ = S // 128         # key tiles per batch
SCALE = 1.0 / float(np.sqrt(HD))

F32 = mybir.dt.float32
BF16 = mybir.dt.bfloat16
Exp = mybir.ActivationFunctionType.Exp
AddOp = mybir.AluOpType.add
MultOp = mybir.AluOpType.mult

_prog_cache = {}
_last_results = {}


def _build_program():
    if "nc" in _prog_cache:
        return _prog_cache["nc"]

    nc = bacc.Bacc("TRN2", target_bir_lowering=False, debug=False,
                   num_devices=NCORES)

    xT = nc.dram_tensor("xT", [D, T], BF16, kind="ExternalInput").ap()
    # column order: k_h0 | k_h1 | q_h0 | q_h1 (128 cols each)
    wqkT = nc.dram_tensor("wqkT", [D, 512], BF16, kind="ExternalInput").ap()
    wvT = nc.dram_tensor("wvT", [D, 256], BF16, kind="ExternalInput").ap()
    bqk_d = nc.dram_tensor("bqk", [128, 4], F32, kind="ExternalInput").ap()
    bqksw_d = nc.dram_tensor("bqksw", [128, 4], F32, kind="ExternalInput").ap()
    bv_d = nc.dram_tensor("bv", [128, 2], F32, kind="ExternalInput").ap()
    cos_d = nc.dram_tensor("cosg", [128, S], BF16, kind="ExternalInput").ap()
    sin_d = nc.dram_tensor("sing", [128, S], BF16, kind="ExternalInput").ap()
    out_d = nc.dram_tensor("out", [256, T], F32, kind="ExternalOutput").ap()

    wqk_src = wqkT.rearrange("(kt p) j -> p kt j", p=128)
    wv_src = wvT.rearrange("(kt p) j -> p kt j", p=128)

    with tile.TileContext(nc) as tc:
        NG = 4                # kt-groups per chunk (DMA/dep granularity)
        GK = NKT // NG        # kt per group

        with tc.tile_pool(name="singles", bufs=1) as singles:
            # weights split into kt-group tiles so consumers only wait on
            # the DMAs of their own group (deps are tracked per tile)
            wk_sb = [singles.tile([128, GK, 256], BF16, name=f"wk{g}")
                     for g in range(NG)]
            wq_sb = [singles.tile([128, GK, 256], BF16, name=f"wq{g}")
                     for g in range(NG)]
            wv_sb = [singles.tile([128, GK, 256], BF16, name=f"wv{g}")
                     for g in range(NG)]
            bqk_sb = singles.tile([128, 4], F32)
            bqksw_sb = singles.tile([128, 4], F32)
            bv_sb = singles.tile([128, 2], F32)
            cos_sb = singles.tile([128, S], BF16)
            sin_sb = singles.tile([128, S], BF16)
            ones_sb = singles.tile([128, 128], BF16)
            nc.vector.memset(ones_sb, 1.0)

            # persistent per-core activations
            # m order: k_h0, k_h1, q_h0, q_h1
            qkT_sb = singles.tile([128, 4, T], BF16)
            v_sb = singles.tile([128, T // 128, 256], BF16)  # v natural

            # ---------------- projection emitters ----------------

            def emit_qk_chain(xc, m, L, pos0, gtok0, wkp, pqp, tag="pq",
                              copy_on_dve=False):
                """One projection chain for m (0=k0,1=k1,2=q0,3=q1) + rope.

                xc is a list of NG kt-group tiles [128, GK, L]."""
                pq = pqp.tile([128, L], F32, name="pq", tag=tag,
                              padded_shape=[128, CHUNK])
                wsb = wk_sb if m < 2 else wq_sb
                col = (m % 2) * 128
                for kt in range(NKT):
                    g, j = kt // GK, kt % GK
                    nc.tensor.matmul(
                        pq, lhsT=wsb[g][:, j, col:col + 128],
                        rhs=xc[g][:, j, 0:L],
                        start=(kt == 0), stop=(kt == NKT - 1))
                raw = wkp.tile([128, L], F32, tag="raw", name="raw",
                               padded_shape=[128, CHUNK])
                if copy_on_dve:
                    nc.vector.tensor_copy(raw, pq)
                else:
                    nc.scalar.copy(raw, pq)
                sw = wkp.tile([128, L], F32, tag="sw", name="sw",
                              padded_shape=[128, CHUNK])
                nc.gpsimd.dma_start(sw[0:64, :], raw[64:128, :])
                nc.gpsimd.dma_start(sw[64:128, :], raw[0:64, :])
                t1 = wkp.tile([128, L], F32, tag="t1", name="t1",
                              padded_shape=[128, CHUNK])
                t2 = wkp.tile([128, L], F32, tag="t2", name="t2",
                              padded_shape=[128, CHUNK])
                nc.vector.scalar_tensor_tensor(
                    t1, raw, bqk_sb[:, m:m + 1], cos_sb[:, pos0:pos0 + L],
                    op0=AddOp, op1=MultOp)
                nc.vector.scalar_tensor_tensor(
                    t2, sw, bqksw_sb[:, m:m + 1], sin_sb[:, pos0:pos0 + L],
                    op0=AddOp, op1=MultOp)
                nc.vector.tensor_add(qkT_sb[:, m, gtok0:gtok0 + L], t1, t2)

            def emit_v(xc, mt0, nmt, gtile0, pvp, tag="pv"):
                """v projection for token tiles [mt0, mt0+nmt) of xc."""
                for i in range(nmt):
                    pv = pvp.tile([128, 256], F32, name="pv", tag=tag,
                                  padded_shape=[128, CHUNK])
                    for kt in range(NKT):
                        g, j = kt // GK, kt % GK
                        nc.tensor.matmul(
                            pv, lhsT=xc[g][:, j, ts(mt0 + i, 128)],
                            rhs=wv_sb[g][:, j, :],
                            start=(kt == 0), stop=(kt == NKT - 1))
                    nc.vector.tensor_copy(v_sb[:, gtile0 + i, :], pv)

            def wsrc(dram, g, c0, c1):
                return dram[g * GK * 128:(g + 1) * GK * 128,
                            c0:c1].rearrange("(j p) c -> p j c", p=128)

            with tc.tile_pool(name="xcp", bufs=2) as xcp, \
                 tc.tile_pool(name="wkp", bufs=2) as wkp:

                def load_xc(tci):
                    xcg = []
                    for g in range(NG):
                        t = xcp.tile([128, GK, CHUNK], BF16, name="xc",
                                     tag="xc", bufs=3 * NG)
                        src = xT[g * GK * 128:(g + 1) * GK * 128,
                                 ts(tci, CHUNK)].rearrange(
                                     "(j p) t -> p j t", p=128)
                        nc.sync.dma_start(t, src)
                        xcg.append(t)
                    return xcg

                # ============ phase 1: chunks 0-3, full qkv ============
                with tc.tile_pool(name="pq1", bufs=3, space="PSUM") as pq1, \
                     tc.tile_pool(name="pv1", bufs=3, space="PSUM") as pv1:

                    # ---- startup DMA stream: the chunk-0-critical data
                    # (wk + x chunk 0) is spread across the sync and scalar
                    # queues; later-needed data (cos/sin, wv, x chunk 1, wq)
                    # streams on gpsimd/scalar behind it ----
                    def xsrc(tci, g):
                        return xT[g * GK * 128:(g + 1) * GK * 128,
                                  ts(tci, CHUNK)].rearrange(
                                      "(j p) t -> p j t", p=128)

                    def new_xc():
                        return xcp.tile([128, GK, CHUNK], BF16, name="xc",
                                        tag="xc", bufs=3 * NG)

                    xc0 = [new_xc() for _ in range(NG)]
                    for g in range(NG):
                        nc.sync.dma_start(wk_sb[g], wsrc(wqkT, g, 0, 256))
                        nc.scalar.dma_start(xc0[g], xsrc(0, g))
                    for g in range(NG):
                        nc.sync.dma_start(wq_sb[g], wsrc(wqkT, g, 256, 512))
                    nc.gpsimd.dma_start(cos_sb[:, 0:1024], cos_d[:, 0:1024])
                    nc.gpsimd.dma_start(sin_sb[:, 0:1024], sin_d[:, 0:1024])
                    nc.gpsimd.dma_start(bqk_sb, bqk_d)
                    nc.gpsimd.dma_start(bqksw_sb, bqksw_d)
                    for g in range(NG):
                        nc.gpsimd.dma_start(wv_sb[g], wsrc(wvT, g, 0, 256))
                    nc.gpsimd.dma_start(bv_sb, bv_d)

                    # chunk 0: all 4 qk chains first (matches DMA arrival
                    # order wk, x0, wq), v last (wv streams in behind)
                    for m in (0, 1, 2, 3):
                        emit_qk_chain(xc0, m, CHUNK, 0, 0, wkp, pq1)
                    emit_v(xc0, 0, 4, 0, pv1)

                    xc1 = load_xc(1)
                    for m in (0, 1):
                        emit_qk_chain(xc1, m, CHUNK, CHUNK, CHUNK, wkp, pq1)
                    emit_v(xc1, 0, 2, 4, pv1)
                    for m in (2, 3):
                        emit_qk_chain(xc1, m, CHUNK, CHUNK, CHUNK, wkp, pq1)
                    emit_v(xc1, 2, 2, 6, pv1)

                    for tci in (2, 3):
                        pos0 = tci * CHUNK
                        xc = load_xc(tci)
                        lo = tci * CHUNK
                        nc.gpsimd.dma_start(cos_sb[:, lo:lo + CHUNK],
                                            cos_d[:, lo:lo + CHUNK])
                        nc.gpsimd.dma_start(sin_sb[:, lo:lo + CHUNK],
                                            sin_d[:, lo:lo + CHUNK])
                        for m in (0, 1):
                            emit_qk_chain(xc, m, CHUNK, pos0, pos0, wkp, pq1)
                        emit_v(xc, 0, 2, tci * 4, pv1)
                        for m in (2, 3):
                            emit_qk_chain(xc, m, CHUNK, pos0, pos0, wkp, pq1)
                        emit_v(xc, 2, 2, tci * 4 + 2, pv1)

                # ======== phases 2+3: rest of proj + all attention ========
                # PSUM: pj2 (2 banks, shared by qk/v chains and the rowsum
                # r tile) + scores (2x2) + o (2) = 8 banks
                with tc.tile_pool(name="pj2", bufs=2, space="PSUM") as pj2, \
                     tc.tile_pool(name="ps_s", bufs=2, space="PSUM") as ps_s, \
                     tc.tile_pool(name="ps_acc", bufs=2, space="PSUM") as ps_acc, \
                     tc.tile_pool(name="ptp", bufs=10) as ptp, \
                     tc.tile_pool(name="rsp", bufs=4) as rsp, \
                     tc.tile_pool(name="aop", bufs=3) as aop:

                    def make_seg_items(b, hl, qc, add_eng=None):
                        """[("u", a_half, b_half)] * 8 + [("c", close)]."""
                        ae = add_eng or nc.vector
                        tok0 = b * S + qc * QCHUNK
                        st = {}

                        def a_half(jj):
                            if jj == 0:
                                st["o"] = ps_acc.tile([128, QCHUNK], F32,
                                                      tag="o", name="o_ps")
                                st["p"] = {}
                            s_ps = ps_s.tile([128, 1024], F32, name="s_ps")
                            for u in (0, 1):
                                j = 2 * jj + u
                                nc.tensor.matmul(
                                    s_ps[:, ts(u, 512)],
                                    lhsT=qkT_sb[:, hl, b * S + j * 128:
                                                b * S + (j + 1) * 128],
                                    rhs=qkT_sb[:, 2 + hl, tok0:tok0 + QCHUNK],
                                    start=True, stop=True)
                            p_sb = ptp.tile([128, 1024], BF16, name="p_sb")
                            nc.scalar.activation(p_sb, s_ps, Exp, scale=SCALE)
                            st["p"][jj] = p_sb

                        def b_half(jj):
                            p_sb = st["p"].pop(jj)
                            for u in (0, 1):
                                j = 2 * jj + u
                                nc.tensor.matmul(
                                    st["o"],
                                    lhsT=v_sb[:, b * NJ + j, ts(hl, 128)],
                                    rhs=p_sb[:, ts(u, 512)],
                                    start=(j == 0), stop=(j == NJ - 1))
                            if jj % 2 == 0:
                                st["pend"] = p_sb
                            elif "acc" not in st:
                                acc = rsp.tile([128, 1024], BF16, tag="acc")
                                ae.tensor_add(acc, st["pend"], p_sb)
                                st["acc"] = acc
                            else:
                                tmp = rsp.tile([128, 1024], BF16, tag="tadd")
                                ae.tensor_add(tmp, st["pend"], p_sb)
                                nacc = rsp.tile([128, 1024], BF16, tag="acc")
                                ae.tensor_add(nacc, st["acc"], tmp)
                                st["acc"] = nacc

                        def close():
                            acc = st.pop("acc")
                            tf = rsp.tile([128, QCHUNK], BF16, tag="tadd")
                            ae.tensor_add(
                                tf, acc[:, 0:512], acc[:, 512:1024])
                            # rowsum r borrows a proj-pool PSUM slot
                            r_ps = pj2.tile([128, QCHUNK], F32, tag="pq",
                                            name="r_ps")
                            nc.tensor.matmul(r_ps, lhsT=ones_sb, rhs=tf,
                                             start=True, stop=True)
                            recip = aop.tile([128, QCHUNK], F32, tag="recip")
                            nc.vector.reciprocal_approx_fast(recip, r_ps)
                            o1 = aop.tile([128, QCHUNK], F32, tag="o1")
                            nc.vector.tensor_mul(o1, st["o"], recip)
                            o2 = aop.tile([128, QCHUNK], F32, tag="o2")
                            nc.vector.tensor_add(
                                o2, o1,
                                bv_sb[:, hl:hl + 1].broadcast_to(
                                    [128, QCHUNK]))
                            nc.sync.dma_start(
                                out_d[ts(hl, 128), tok0:tok0 + QCHUNK], o2)

                        items = []
                        for jj in range(NJ // 2):
                            items.append(("u",
                                          lambda jj=jj: a_half(jj),
                                          lambda jj=jj: b_half(jj)))
                        items.append(("c", close))
                        return items

                    fifo = deque()
                    pend_b = [None]
                    pend_close = deque()  # [close_thunk, units_to_wait]

                    def emit_n(n):
                        while n > 0 and fifo:
                            it = fifo.popleft()
                            if it[0] == "c":
                                pend_close.append([it[1], 2])
                                continue
                            it[1]()                      # scores+exp, unit n
                            if pend_b[0] is not None:
                                pend_b[0]()              # pv of unit n-1
                            pend_b[0] = it[2]
                            n -= 1
                            for pc in pend_close:
                                pc[1] -= 1
                            while pend_close and pend_close[0][1] <= 0:
                                if pend_b[0] is not None:
                                    pend_b[0]()
                                    pend_b[0] = None
                                pend_close.popleft()[0]()

                    def drain():
                        if pend_b[0] is not None:
                            pend_b[0]()
                            pend_b[0] = None
                        while pend_close:
                            pend_close.popleft()[0]()

                    # phase 2: chunks 4-7, k+v only, all batch-0 attn
                    # interleaved (18 units per chunk)
                    for si in range(8):
                        fifo.extend(make_seg_items(0, si % 2, si // 2))
                    for tci in range(4, 8):
                        pos0 = (tci % 4) * CHUNK
                        gtok0 = tci * CHUNK
                        xc = load_xc(tci)
                        emit_qk_chain(xc, 0, CHUNK, pos0, gtok0, wkp, pj2)
                        emit_n(4)
                        emit_qk_chain(xc, 1, CHUNK, pos0, gtok0, wkp, pj2)
                        emit_n(4)
                        emit_v(xc, 0, 2, tci * 4, pj2, tag="pq")
                        emit_n(4)
                        emit_v(xc, 2, 2, tci * 4 + 2, pj2, tag="pq")
                        emit_n(4)

                    # first two deferred q chains emitted at the end of
                    # phase 2: their ropes complete while the remaining
                    # batch-0 units bridge, so phase-3 scores never wait
                    chains = [(tci, hl) for tci in range(4, 8)
                              for hl in range(2)]
                    xc_q = [load_xc(4)]
                    for hl in range(2):
                        emit_qk_chain(xc_q[0], 2 + hl, CHUNK, 0, 4 * CHUNK,
                                      wkp, pj2)
                        emit_n(3)

                    # phase 3: remaining q chains + batch-1 attn; chains run
                    # two ahead of the seg they unlock.  Projection-chain
                    # PSUM copies go to DVE here: ACT is exp-saturated.
                    for i, (tci, hl) in enumerate(chains[2:]):
                        pos0 = (tci % 4) * CHUNK
                        gtok0 = tci * CHUNK
                        if hl == 0:
                            xc_q[0] = load_xc(tci)
                        emit_qk_chain(xc_q[0], 2 + hl, CHUNK, pos0, gtok0,
                                      wkp, pj2, copy_on_dve=True)
                        ptci, phl = chains[i]
                        fifo.extend(make_seg_items(1, phl, ptci % 4,
                                                   add_eng=nc.gpsimd))
                        emit_n(9)
                    for (tci, hl) in chains[-2:]:
                        fifo.extend(make_seg_items(1, hl, tci % 4,
                                                   add_eng=nc.gpsimd))
                        emit_n(9)
                    while fifo:
                        emit_n(9)
                    drain()

    nc.compile()
    _prog_cache["nc"] = nc
    return nc


_PERM = np.concatenate([np.arange(0, 128, 2), np.arange(1, 128, 2)])


def _prep_inputs(sequence, frequencies, Wq, bq, Wk, bk, Wv, bv):
    bf = ml_dtypes.bfloat16
    x = np.ascontiguousarray(sequence.reshape(T, D))
    xT = np.ascontiguousarray(x.T).astype(bf)

    i_idx = np.arange(128) % 64
    ang = np.asarray(frequencies, np.float32)
    cos_g = np.ascontiguousarray(np.cos(ang[:, i_idx]).T)
    sin_g = np.ascontiguousarray(np.sin(ang[:, i_idx]).T)
    sin_g[:64] *= -1.0
    cos_g = cos_g.astype(bf)
    sin_g = sin_g.astype(bf)

    in_maps = []
    for c in range(NCORES):
        h0, h1 = 2 * c, 2 * c + 1
        # column order: k_h0 | k_h1 | q_h0 | q_h1
        WQK = np.concatenate(
            [Wk[h * 128:(h + 1) * 128][_PERM] for h in (h0, h1)]
            + [Wq[h * 128:(h + 1) * 128][_PERM] for h in (h0, h1)], 0)
        bqk = np.concatenate(
            [bk[h * 128:(h + 1) * 128][_PERM] for h in (h0, h1)]
            + [bq[h * 128:(h + 1) * 128][_PERM] for h in (h0, h1)])
        WV = np.concatenate([Wv[h * 128:(h + 1) * 128] for h in (h0, h1)], 0)
        bvc = np.concatenate([bv[h * 128:(h + 1) * 128] for h in (h0, h1)])
        in_maps.append({
            "xT": xT,
            "wqkT": np.ascontiguousarray(WQK.T).astype(bf),
            "wvT": np.ascontiguousarray(WV.T).astype(bf),
            "bqk": np.ascontiguousarray(bqk.reshape(4, 128).T).astype(np.float32),
            "bqksw": np.ascontiguousarray(
                np.roll(bqk.reshape(4, 128), 64, axis=1).T).astype(np.float32),
            "bv": np.ascontiguousarray(bvc.reshape(2, 128).T).astype(np.float32),
            "cosg": cos_g,
            "sing": sin_g,
        })
    return in_maps


def kernel(sequence, frequencies, mask, Wq, bq, Wk, bk, Wv, bv):
    sequence = np.asarray(sequence, np.float32)
    frequencies = np.asarray(frequencies, np.float32)
    Wq, bq = np.asarray(Wq, np.float32), np.asarray(bq, np.float32)
    Wk, bk = np.asarray(Wk, np.float32), np.asarray(bk, np.float32)
    Wv, bv = np.asarray(Wv, np.float32), np.asarray(bv, np.float32)
    nc = _build_program()
    in_maps = _prep_inputs(sequence, frequencies, Wq, bq, Wk, bk, Wv, bv)
    trace = bool(int(os.environ.get("BENCH_TRACE", "0")))
    res = run_bass_kernel_spmd(nc, in_maps, list(range(NCORES)), trace=trace)
    _last_results["exec_time_ns"] = res.exec_time_ns
    _last_results["results"] = res

    out = np.empty((B, S, D), np.float32)
    for c in range(NCORES):
        oc = res.results[c]["out"]           # [256, T]
        for hl in range(2):
            h = 2 * c + hl
            for b in range(B):
                out[b, :, h * 128:(h + 1) * 128] = \
                    oc[hl * 128:(hl + 1) * 128, b * S:(b + 1) * S].T
    return out


# revision 49
# speedup vs baseline: 1.2386x; 1.2386x over previous
"""Trainium2 Bass kernel for nn_Attention_33595234189924.

Multi-head attention (B=2, S=2048, D=2048, H=16, hd=128) with RoPE,
tensor-parallel over heads: 8 cores x 2 heads each.

v4 schedule, three phases:
  1. chunks 0-3 (batch 0): full q/k/v projections, PE-dense; chunk 0 is
     split in two 256-token halves with per-kt interleaved weight/x DMAs
     for fast rampup.
  2. chunks 4-7 (batch 1): k+v projections only, with batch-0 attention
     segments (fully unlocked after phase 1) interleaved at ~18 units
     per chunk.  The Tile list-scheduler slots attention matmuls into
     projection-chain bubbles.
  3. deferred q chains for chunks 4-7 (both heads), each immediately
     followed by the batch-1 attention segment it unlocks.  This keeps
     ~27us of PE projection work in the final ACT-heavy region so the
     PE never starves while exp runs.
  PSUM in phases 2-3: proj(1 bank) + scores(2x2) + o(2) + r(1) = 8.

Per-core dataflow (activations in [feature, token] layout):
  - k/q projections -> PSUM -> +bias -> RoPE (rotate-half via host-side
    even/odd weight-row permutation + 64-partition block swap by DMA)
  - v projection in natural [token, hd] layout (xT tiles as stationary)
  - scores^T = k_tile^T @ q  per 128-key tile, exp on ACT (scale fused),
    probs kept transposed -> PV accumulates in PSUM; row-sums via
    all-ones stationary matmul (output pre-broadcast across partitions)
  - out = PV/rowsum + bv  (v-bias folded through softmax identity)
"""

import os
import sys
from collections import deque

sys.path.insert(0, "/opt/trn_rl_repo")

import numpy as np
import ml_dtypes

import concourse.bass as bass
import concourse.tile as tile
from concourse import bacc, mybir
from concourse.bass import ts
from concourse.bass_utils import run_bass_kernel_spmd

# If anything enables tracing (e.g. BASS_TRACE in the environment) and the
# image's antenv lacks axon_hooks, run_bass_kernel_spmd would crash on
# import. Register a null hook so it degrades to the untraced path.
try:
    from antenv import axon_hooks as _ah  # noqa: F401
except Exception:
    import types as _types

    _m = _types.ModuleType("antenv.axon_hooks")
    _m.get_axon_ntff_profile_hook = lambda: None
    _m.set_axon_ntff_profile_hook = lambda hook: None
    sys.modules["antenv.axon_hooks"] = _m

B, S, D, H = 2, 2048, 2048, 16
HD = 128
T = B * S
NCORES = 8
NKT = D // 128        # contraction tiles for projections
CHUNK = 512           # token chunk in projection phase
QCHUNK = 512          # query chunk in attention phase
NJ = S // 128         # key tiles per batch
SCALE = 1.0 / float(np.sqrt(HD))

F32 = mybir.dt.float32
BF16 = mybir.dt.bfloat16
Exp = mybir.ActivationFunctionType.Exp
AddOp = mybir.AluOpType.add
MultOp = mybir.AluOpType.mult

_prog_cache = {}
_last_results = {}


def _build_program():
    if "nc" in _prog_cache:
        return _prog_cache["nc"]

    nc = bacc.Bacc("TRN2", target_bir_lowering=False, debug=False,
                   num_devices=NCORES)

    xT = nc.dram_tensor("xT", [D, T], BF16, kind="ExternalInput").ap()
    # column order: k_h0 | k_h1 | q_h0 | q_h1 (128 cols each)
    wqkT = nc.dram_tensor("wqkT", [D, 512], BF16, kind="ExternalInput").ap()
    wvT = nc.dram_tensor("wvT", [D, 256], BF16, kind="ExternalInput").ap()
    bqk_d = nc.dram_tensor("bqk", [128, 4], F32, kind="ExternalInput").ap()
    bqksw_d = nc.dram_tensor("bqksw", [128, 4], F32, kind="ExternalInput").ap()
    bv_d = nc.dram_tensor("bv", [128, 2], F32, kind="ExternalInput").ap()
    cos_d = nc.dram_tensor("cosg", [128, S], BF16, kind="ExternalInput").ap()
    sin_d = nc.dram_tensor("sing", [128, S], BF16, kind="ExternalInput").ap()
    out_d = nc.dram_tensor("out", [256, T], F32, kind="ExternalOutput").ap()

    wqk_src = wqkT.rearrange("(kt p) j -> p kt j", p=128)
    wv_src = wvT.rearrange("(kt p) j -> p kt j", p=128)

    with tile.TileContext(nc) as tc:
        NG = 4                # kt-groups per chunk (DMA/dep granularity)
        GK = NKT // NG        # kt per group

        with tc.tile_pool(name="singles", bufs=1) as singles:
            # weights split into kt-group tiles so consumers only wait on
            # the DMAs of their own group (deps are tracked per tile)
            wk_sb = [singles.tile([128, GK, 256], BF16, name=f"wk{g}")
                     for g in range(NG)]
            wq_sb = [singles.tile([128, GK, 256], BF16, name=f"wq{g}")
                     for g in range(NG)]
            wv_sb = [singles.tile([128, GK, 256], BF16, name=f"wv{g}")
                     for g in range(NG)]
            bqk_sb = singles.tile([128, 4], F32)
            bqksw_sb = singles.tile([128, 4], F32)
            bv_sb = singles.tile([128, 2], F32)
            cos_sb = singles.tile([128, S], BF16)
            sin_sb = singles.tile([128, S], BF16)
            ones_sb = singles.tile([128, 128], BF16)
            nc.vector.memset(ones_sb, 1.0)

            # persistent per-core activations
            # m order: k_h0, k_h1, q_h0, q_h1
            qkT_sb = singles.tile([128, 4, T], BF16)
            v_sb = singles.tile([128, T // 128, 256], BF16)  # v natural

            # ---------------- projection emitters ----------------

            def emit_qk_chain(xc, m, L, pos0, gtok0, wkp, pqp, tag="pq",
                              copy_on_dve=False):
                """One projection chain for m (0=k0,1=k1,2=q0,3=q1) + rope.

                xc is a list of NG kt-group tiles [128, GK, L]."""
                pq = pqp.tile([128, L], F32, name="pq", tag=tag,
                              padded_shape=[128, CHUNK])
                wsb = wk_sb if m < 2 else wq_sb
                col = (m % 2) * 128
                for kt in range(NKT):
                    g, j = kt // GK, kt % GK
                    nc.tensor.matmul(
                        pq, lhsT=wsb[g][:, j, col:col + 128],
                        rhs=xc[g][:, j, 0:L],
                        start=(kt == 0), stop=(kt == NKT - 1))
                raw = wkp.tile([128, L], F32, tag="raw", name="raw",
                               padded_shape=[128, CHUNK])
                if copy_on_dve:
                    nc.vector.tensor_copy(raw, pq)
                else:
                    nc.scalar.copy(raw, pq)
                sw = wkp.tile([128, L], F32, tag="sw", name="sw",
                              padded_shape=[128, CHUNK])
                nc.gpsimd.dma_start(sw[0:64, :], raw[64:128, :])
                nc.gpsimd.dma_start(sw[64:128, :], raw[0:64, :])
                t1 = wkp.tile([128, L], F32, tag="t1", name="t1",
                              padded_shape=[128, CHUNK])
                t2 = wkp.tile([128, L], F32, tag="t2", name="t2",
                              padded_shape=[128, CHUNK])
                nc.vector.scalar_tensor_tensor(
                    t1, raw, bqk_sb[:, m:m + 1], cos_sb[:, pos0:pos0 + L],
                    op0=AddOp, op1=MultOp)
                nc.vector.scalar_tensor_tensor(
                    t2, sw, bqksw_sb[:, m:m + 1], sin_sb[:, pos0:pos0 + L],
                    op0=AddOp, op1=MultOp)
                nc.vector.tensor_add(qkT_sb[:, m, gtok0:gtok0 + L], t1, t2)

            def emit_v(xc, mt0, nmt, gtile0, pvp, tag="pv"):
                """v projection for token tiles [mt0, mt0+nmt) of xc."""
                for i in range(nmt):
                    pv = pvp.tile([128, 256], F32, name="pv", tag=tag,
                                  padded_shape=[128, CHUNK])
                    for kt in range(NKT):
                        g, j = kt // GK, kt % GK
                        nc.tensor.matmul(
                            pv, lhsT=xc[g][:, j, ts(mt0 + i, 128)],
                            rhs=wv_sb[g][:, j, :],
                            start=(kt == 0), stop=(kt == NKT - 1))
                    nc.vector.tensor_copy(v_sb[:, gtile0 + i, :], pv)

            def wsrc(dram, g, c0, c1):
                return dram[g * GK * 128:(g + 1) * GK * 128,
                            c0:c1].rearrange("(j p) c -> p j c", p=128)

            with tc.tile_pool(name="xcp", bufs=2) as xcp, \
                 tc.tile_pool(name="wkp", bufs=2) as wkp:

                def load_xc(tci):
                    xcg = []
                    for g in range(NG):
                        t = xcp.tile([128, GK, CHUNK], BF16, name="xc",
                                     tag="xc", bufs=3 * NG)
                        src = xT[g * GK * 128:(g + 1) * GK * 128,
                                 ts(tci, CHUNK)].rearrange(
                                     "(j p) t -> p j t", p=128)
                        nc.sync.dma_start(t, src)
                        xcg.append(t)
                    return xcg

                # ============ phase 1: chunks 0-3, full qkv ============
                with tc.tile_pool(name="pq1", bufs=3, space="PSUM") as pq1, \
                     tc.tile_pool(name="pv1", bufs=3, space="PSUM") as pv1:

                    # ---- startup DMA stream: the chunk-0-critical data
                    # (wk + x chunk 0) is spread across the sync and scalar
                    # queues; later-needed data (cos/sin, wv, x chunk 1, wq)
                    # streams on gpsimd/scalar behind it ----
                    def xsrc(tci, g):
                        return xT[g * GK * 128:(g + 1) * GK * 128,
                                  ts(tci, CHUNK)].rearrange(
                                      "(j p) t -> p j t", p=128)

                    def new_xc():
                        return xcp.tile([128, GK, CHUNK], BF16, name="xc",
                                        tag="xc", bufs=3 * NG)

                    xc0 = [new_xc() for _ in range(NG)]
                    for g in range(NG):
                        nc.sync.dma_start(wk_sb[g], wsrc(wqkT, g, 0, 256))
                        nc.scalar.dma_start(xc0[g], xsrc(0, g))
                    for g in range(NG):
                        nc.sync.dma_start(wq_sb[g], wsrc(wqkT, g, 256, 512))
                    nc.gpsimd.dma_start(cos_sb[:, 0:1024], cos_d[:, 0:1024])
                    nc.gpsimd.dma_start(sin_sb[:, 0:1024], sin_d[:, 0:1024])
                    nc.gpsimd.dma_start(bqk_sb, bqk_d)
                    nc.gpsimd.dma_start(bqksw_sb, bqksw_d)
                    for g in range(NG):
                        nc.gpsimd.dma_start(wv_sb[g], wsrc(wvT, g, 0, 256))
                    nc.gpsimd.dma_start(bv_sb, bv_d)

                    # chunk 0: all 4 qk chains first (matches DMA arrival
                    # order wk, x0, wq), v last (wv streams in behind)
                    for m in (0, 1, 2, 3):
                        emit_qk_chain(xc0, m, CHUNK, 0, 0, wkp, pq1)
                    emit_v(xc0, 0, 4, 0, pv1)

                    xc1 = load_xc(1)
                    for m in (0, 1):
                        emit_qk_chain(xc1, m, CHUNK, CHUNK, CHUNK, wkp, pq1)
                    emit_v(xc1, 0, 2, 4, pv1)
                    for m in (2, 3):
                        emit_qk_chain(xc1, m, CHUNK, CHUNK, CHUNK, wkp, pq1)
                    emit_v(xc1, 2, 2, 6, pv1)

                    for tci in (2, 3):
                        pos0 = tci * CHUNK
                        xc = load_xc(tci)
                        lo = tci * CHUNK
                        nc.gpsimd.dma_start(cos_sb[:, lo:lo + CHUNK],
                                            cos_d[:, lo:lo + CHUNK])
                        nc.gpsimd.dma_start(sin_sb[:, lo:lo + CHUNK],
                                            sin_d[:, lo:lo + CHUNK])
                        for m in (0, 1):
                            emit_qk_chain(xc, m, CHUNK, pos0, pos0, wkp, pq1)
                        emit_v(xc, 0, 2, tci * 4, pv1)
                        for m in (2, 3):
                            emit_qk_chain(xc, m, CHUNK, pos0, pos0, wkp, pq1)
                        emit_v(xc, 2, 2, tci * 4 + 2, pv1)

                # ======== phases 2+3: rest of proj + all attention ========
                # PSUM: pj2 (2 banks, shared by qk/v chains and the rowsum
                # r tile) + scores (2x2) + o (2) = 8 banks
                with tc.tile_pool(name="pj2", bufs=2, space="PSUM") as pj2, \
                     tc.tile_pool(name="ps_s", bufs=2, space="PSUM") as ps_s, \
                     tc.tile_pool(name="ps_acc", bufs=2, space="PSUM") as ps_acc, \
                     tc.tile_pool(name="ptp", bufs=10) as ptp, \
                     tc.tile_pool(name="rsp", bufs=4) as rsp, \
                     tc.tile_pool(name="aop", bufs=3) as aop:

                    def make_seg_items(b, hl, qc, add_eng=None):
                        """[("u", a_half, b_half)] * 8 + [("c", close)]."""
                        ae = add_eng or nc.vector
                        tok0 = b * S + qc * QCHUNK
                        st = {}

                        def a_half(jj):
                            if jj == 0:
                                st["o"] = ps_acc.tile([128, QCHUNK], F32,
                                                      tag="o", name="o_ps")
                                st["p"] = {}
                            s_ps = ps_s.tile([128, 1024], F32, name="s_ps")
                            for u in (0, 1):
                                j = 2 * jj + u
                                nc.tensor.matmul(
                                    s_ps[:, ts(u, 512)],
                                    lhsT=qkT_sb[:, hl, b * S + j * 128:
                                                b * S + (j + 1) * 128],
                                    rhs=qkT_sb[:, 2 + hl, tok0:tok0 + QCHUNK],
                                    start=True, stop=True)
                            p_sb = ptp.tile([128, 1024], BF16, name="p_sb")
                            nc.scalar.activation(p_sb, s_ps, Exp, scale=SCALE)
                            st["p"][jj] = p_sb

                        def b_half(jj):
                            p_sb = st["p"].pop(jj)
                            for u in (0, 1):
                                j = 2 * jj + u
                                nc.tensor.matmul(
                                    st["o"],
                                    lhsT=v_sb[:, b * NJ + j, ts(hl, 128)],
                                    rhs=p_sb[:, ts(u, 512)],
                                    start=(j == 0), stop=(j == NJ - 1))
                            if jj % 2 == 0:
                                st["pend"] = p_sb
                            elif "acc" not in st:
                                acc = rsp.tile([128, 1024], BF16, tag="acc")
                                ae.tensor_add(acc, st["pend"], p_sb)
                                st["acc"] = acc
                            else:
                                tmp = rsp.tile([128, 1024], BF16, tag="tadd")
                                ae.tensor_add(tmp, st["pend"], p_sb)
                                nacc = rsp.tile([128, 1024], BF16, tag="acc")
                                ae.tensor_add(nacc, st["acc"], tmp)
                                st["acc"] = nacc

                        def close():
                            acc = st.pop("acc")
                            tf = rsp.tile([128, QCHUNK], BF16, tag="tadd")
                            ae.tensor_add(
                                tf, acc[:, 0:512], acc[:, 512:1024])
                            # rowsum r borrows a proj-pool PSUM slot
                            r_ps = pj2.tile([128, QCHUNK], F32, tag="pq",
                                            name="r_ps")
                            nc.tensor.matmul(r_ps, lhsT=ones_sb, rhs=tf,
                                             start=True, stop=True)
                            recip = aop.tile([128, QCHUNK], F32, tag="recip")
                            nc.vector.reciprocal_approx_fast(recip, r_ps)
                            o1 = aop.tile([128, QCHUNK], F32, tag="o1")
                            nc.vector.tensor_mul(o1, st["o"], recip)
                            o2 = aop.tile([128, QCHUNK], F32, tag="o2")
                            nc.vector.tensor_add(
                                o2, o1,
                                bv_sb[:, hl:hl + 1].broadcast_to(
                                    [128, QCHUNK]))
                            nc.sync.dma_start(
                                out_d[ts(hl, 128), tok0:tok0 + QCHUNK], o2)

                        items = []
                        for jj in range(NJ // 2):
                            items.append(("u",
                                          lambda jj=jj: a_half(jj),
                                          lambda jj=jj: b_half(jj)))
                        items.append(("c", close))
                        return items

                    fifo = deque()
                    pend_b = [None]
                    pend_close = deque()  # [close_thunk, units_to_wait]

                    def emit_n(n):
                        while n > 0 and fifo:
                            it = fifo.popleft()
                            if it[0] == "c":
                                pend_close.append([it[1], 2])
                                continue
                            it[1]()                      # scores+exp, unit n
                            if pend_b[0] is not None:
                                pend_b[0]()              # pv of unit n-1
                            pend_b[0] = it[2]
                            n -= 1
                            for pc in pend_close:
                                pc[1] -= 1
                            while pend_close and pend_close[0][1] <= 0:
                                if pend_b[0] is not None:
                                    pend_b[0]()
                                    pend_b[0] = None
                                pend_close.popleft()[0]()

                    def drain():
                        if pend_b[0] is not None:
                            pend_b[0]()
                            pend_b[0] = None
                        while pend_close:
                            pend_close.popleft()[0]()

                    # phase 2: chunks 4-7, k+v only, all batch-0 attn
                    # interleaved (18 units per chunk)
                    for si in range(8):
                        fifo.extend(make_seg_items(0, si % 2, si // 2))
                    for tci in range(4, 8):
                        pos0 = (tci % 4) * CHUNK
                        gtok0 = tci * CHUNK
                        xc = load_xc(tci)
                        emit_qk_chain(xc, 0, CHUNK, pos0, gtok0, wkp, pj2)
                        emit_n(4)
                        emit_qk_chain(xc, 1, CHUNK, pos0, gtok0, wkp, pj2)
                        emit_n(4)
                        emit_v(xc, 0, 2, tci * 4, pj2, tag="pq")
                        emit_n(4)
                        emit_v(xc, 2, 2, tci * 4 + 2, pj2, tag="pq")
                        emit_n(4)

                    # first two deferred q chains emitted at the end of
                    # phase 2: their ropes complete while the remaining
                    # batch-0 units bridge, so phase-3 scores never wait
                    chains = [(tci, hl) for tci in range(4, 8)
                              for hl in range(2)]
                    xc_q = [load_xc(4)]
                    for hl in range(2):
                        emit_qk_chain(xc_q[0], 2 + hl, CHUNK, 0, 4 * CHUNK,
                                      wkp, pj2)
                        emit_n(3)

                    # phase 3: remaining q chains + batch-1 attn; chains run
                    # two ahead of the seg they unlock.  Projection-chain
                    # PSUM copies go to DVE here: ACT is exp-saturated.
                    for i, (tci, hl) in enumerate(chains[2:]):
                        pos0 = (tci % 4) * CHUNK
                        gtok0 = tci * CHUNK
                        if hl == 0:
                            xc_q[0] = load_xc(tci)
                        emit_qk_chain(xc_q[0], 2 + hl, CHUNK, pos0, gtok0,
                                      wkp, pj2, copy_on_dve=True)
                        ptci, phl = chains[i]
                        fifo.extend(make_seg_items(1, phl, ptci % 4))
                        emit_n(9)
                    for (tci, hl) in chains[-2:]:
                        fifo.extend(make_seg_items(1, hl, tci % 4))
                        emit_n(9)
                    while fifo:
                        emit_n(9)
                    drain()

    nc.compile()
    _prog_cache["nc"] = nc
    return nc


_PERM = np.concatenate([np.arange(0, 128, 2), np.arange(1, 128, 2)])


def _prep_inputs(sequence, frequencies, Wq, bq, Wk, bk, Wv, bv):
    bf = ml_dtypes.bfloat16
    x = np.ascontiguousarray(sequence.reshape(T, D))
    xT = np.ascontiguousarray(x.T).astype(bf)

    i_idx = np.arange(128) % 64
    ang = np.asarray(frequencies, np.float32)
    cos_g = np.ascontiguousarray(np.cos(ang[:, i_idx]).T)
    sin_g = np.ascontiguousarray(np.sin(ang[:, i_idx]).T)
    sin_g[:64] *= -1.0
    cos_g = cos_g.astype(bf)
    sin_g = sin_g.astype(bf)

    in_maps = []
    for c in range(NCORES):
        h0, h1 = 2 * c, 2 * c + 1
        # column order: k_h0 | k_h1 | q_h0 | q_h1
        WQK = np.concatenate(
            [Wk[h * 128:(h + 1) * 128][_PERM] for h in (h0, h1)]
            + [Wq[h * 128:(h + 1) * 128][_PERM] for h in (h0, h1)], 0)
        bqk = np.concatenate(
            [bk[h * 128:(h + 1) * 128][_PERM] for h in (h0, h1)]
            + [bq[h * 128:(h + 1) * 128][_PERM] for h in (h0, h1)])
        WV = np.concatenate([Wv[h * 128:(h + 1) * 128] for h in (h0, h1)], 0)
        bvc = np.concatenate([bv[h * 128:(h + 1) * 128] for h in (h0, h1)])
        in_maps.append({
            "xT": xT,
            "wqkT": np.ascontiguousarray(WQK.T).astype(bf),
            "wvT": np.ascontiguousarray(WV.T).astype(bf),
            "bqk": np.ascontiguousarray(bqk.reshape(4, 128).T).astype(np.float32),
            "bqksw": np.ascontiguousarray(
                np.roll(bqk.reshape(4, 128), 64, axis=1).T).astype(np.float32),
            "bv": np.ascontiguousarray(bvc.reshape(2, 128).T).astype(np.float32),
            "cosg": cos_g,
            "sing": sin_g,
        })
    return in_maps


def kernel(sequence, frequencies, mask, Wq, bq, Wk, bk, Wv, bv):
    sequence = np.asarray(sequence, np.float32)
    frequencies = np.asarray(frequencies, np.float32)
    Wq, bq = np.asarray(Wq, np.float32), np.asarray(bq, np.float32)
    Wk, bk = np.asarray(Wk, np.float32), np.asarray(bk, np.float32)
    Wv, bv = np.asarray(Wv, np.float32), np.asarray(bv, np.float32)
    nc = _build_program()
    in_maps = _prep_inputs(sequence, frequencies, Wq, bq, Wk, bk, Wv, bv)
    trace = bool(int(os.environ.get("BENCH_TRACE", "0")))
    res = run_bass_kernel_spmd(nc, in_maps, list(range(NCORES)), trace=trace)
    _last_results["exec_time_ns"] = res.exec_time_ns
    _last_results["results"] = res

    out = np.empty((B, S, D), np.float32)
    for c in range(NCORES):
        oc = res.results[c]["out"]           # [256, T]
        for hl in range(2):
            h = 2 * c + hl
            for b in range(B):
                out[b, :, h * 128:(h + 1) * 128] = \
                    oc[hl * 128:(hl + 1) * 128, b * S:(b + 1) * S].T
    return out
